# revision 7
# baseline (speedup 1.0000x reference)
"""Trainium2 Bass kernel for nn_SNSCell (gnn_message_passing).

Math (per batch row b, feature j, n=128):
    Gm,bm,Gmax,Esyn are clipped; ge[j] = sum_i Gmax[i,j]*Esyn[i,j]
    P = h @ Gmax
    out[b,j] = (1-Gm[j])*h[b,j] + bm[j] + i_app[b,j]
               + clamp01(h[b,j]) * (ge[j] - P[b,j])

Strategy: data-parallel over batch across 8 cores (32768 rows each).
On-chip, work in the transposed domain (features on partitions) so
per-feature params are per-partition scalars and the matmul keeps a
fixed weight orientation:
  - h loaded with fp32->bf16 cast during DMA (SWDGE)
  - PE-transpose bf16 h blocks [128b,128i] -> hT [128i,128b]
  - PSUM Q = ge (K=2 hi/lo seed matmul) - Gmax^T-contraction with hT
  - elementwise on DVE (clamp/mul/fused scale-add), (1-Gm)/bm folded in
  - PE-transpose result back (bf16), add i_app in natural fp32, store.
"""

import numpy as np
import ml_dtypes
from contextlib import ExitStack

import concourse.bacc as bacc
import concourse.tile as tile
from concourse import mybir
from concourse.bass_utils import run_bass_kernel_spmd

B_FULL = 262144
N = 128
N_CORES = 8
ROWS = B_FULL // N_CORES          # 32768 rows per core
RPP = 16                          # rows packed per partition per DMA chunk
CHUNK_COLS = RPP * N              # 2048 cols per chunk tile
N_CHUNKS = ROWS // (128 * RPP)    # 16 chunks of [128, 2048] (1 MiB fp32)
SUPER = 512                       # cols per compute super-tile (= PSUM bank)
N_SUPER = CHUNK_COLS // SUPER     # 4 super-tiles per chunk

F32 = mybir.dt.float32
BF16 = mybir.dt.bfloat16
AOT = mybir.AluOpType
ACT_F = mybir.ActivationFunctionType
BF = ml_dtypes.bfloat16

_CACHE = {}


def _build():
    nc = bacc.Bacc("TRN2", debug=False)

    h = nc.dram_tensor("h", [ROWS, N], F32, kind="ExternalInput").ap()
    ia = nc.dram_tensor("ia", [ROWS, N], F32, kind="ExternalInput").ap()
    negG = nc.dram_tensor("negG", [N, N], BF16, kind="ExternalInput").ap()
    identb = nc.dram_tensor("identb", [N, N], BF16, kind="ExternalInput").ap()
    ge = nc.dram_tensor("ge", [N, 1], F32, kind="ExternalInput").ap()
    omg = nc.dram_tensor("omg", [N, 1], F32, kind="ExternalInput").ap()
    bm = nc.dram_tensor("bm", [N, 1], F32, kind="ExternalInput").ap()
    out = nc.dram_tensor("out", [ROWS, N], F32, kind="ExternalOutput").ap()

    hv = h.rearrange("(n p r) m -> n p (r m)", p=128, r=RPP)
    iav = ia.rearrange("(n p r) m -> n p (r m)", p=128, r=RPP)
    outv = out.rearrange("(n p r) m -> n p (r m)", p=128, r=RPP)

    with tile.TileContext(nc) as tc:
        with ExitStack() as ctx:
            const = ctx.enter_context(tc.tile_pool(name="const", bufs=1))
            io = ctx.enter_context(tc.tile_pool(name="io", bufs=2))
            mid = ctx.enter_context(tc.tile_pool(name="mid", bufs=3))
            ps = ctx.enter_context(tc.tile_pool(name="ps", bufs=2, space="PSUM"))

            negG_s = const.tile([N, N], BF16, tag="negG")
            ident_s = const.tile([N, N], BF16, tag="ident")
            ge_s = const.tile([N, 1], F32, tag="ge")
            omg_s = const.tile([N, 1], F32, tag="omg")
            bm_s = const.tile([N, 1], F32, tag="bm")
            nc.sync.dma_start(negG_s[:], negG[:])
            nc.sync.dma_start(ident_s[:], identb[:])
            nc.sync.dma_start(ge_s[:], ge[:])
            nc.sync.dma_start(omg_s[:], omg[:])
            nc.sync.dma_start(bm_s[:], bm[:])

            for n in range(N_CHUNKS):
                hb = io.tile([128, CHUNK_COLS], BF16, tag="hb")
                iac = io.tile([128, CHUNK_COLS], BF16, tag="iac")
                oc = io.tile([128, CHUNK_COLS], F32, tag="oc")
                # cast fp32 -> bf16 during the load (SWDGE)
                nc.gpsimd.dma_start(hb[:], hv[n])
                nc.gpsimd.dma_start(iac[:], iav[n])

                for s in range(N_SUPER):
                    sl = slice(s * SUPER, (s + 1) * SUPER)

                    # hT blocks: [128 i, 512 b], bf16
                    T = ps.tile([128, SUPER], BF16, tag="T")
                    for r in range(4):
                        c0 = s * SUPER + r * 128
                        nc.tensor.transpose(
                            T[:, r * 128 : (r + 1) * 128],
                            hb[:, c0 : c0 + 128],
                            ident_s[:],
                        )
                    ht = mid.tile([128, SUPER], BF16, tag="ht")
                    nc.scalar.copy(ht[:], T[:])

                    # Q = -P^T
                    Q = ps.tile([128, SUPER], F32, tag="Q")
                    nc.tensor.matmul(Q[:], negG_s[:], ht[:], start=True, stop=True)

                    # t1 = ge - P^T   (ACT, PSUM src, per-partition bias)
                    t1 = mid.tile([128, SUPER], BF16, tag="t1")
                    nc.scalar.activation(
                        t1[:], Q[:], ACT_F.Identity, bias=ge_s[:], scale=1.0
                    )
                    # c = clamp01(hT)
                    cl = mid.tile([128, SUPER], BF16, tag="cl")
                    nc.vector.tensor_scalar(
                        cl[:], ht[:], 0.0, 1.0, AOT.max, AOT.min
                    )
                    # t = c * (ge - P^T)
                    t = mid.tile([128, SUPER], BF16, tag="t")
                    nc.vector.tensor_mul(t[:], cl[:], t1[:])
                    # rr = (1-Gm)*hT + bm   (DVE dual-scalar tensor_scalar, 4x)
                    rr = mid.tile([128, SUPER], BF16, tag="rr")
                    nc.vector.tensor_scalar(
                        rr[:], ht[:], omg_s[:], bm_s[:], AOT.mult, AOT.add
                    )
                    # u = rr + t
                    u = mid.tile([128, SUPER], BF16, tag="u")
                    nc.vector.tensor_add(u[:], rr[:], t[:])

                    # transpose back to natural layout
                    OT = ps.tile([128, SUPER], BF16, tag="OT")
                    for r in range(4):
                        rs = slice(r * 128, (r + 1) * 128)
                        nc.tensor.transpose(OT[:, rs], u[:, rs], ident_s[:])

                    # out = OT + i_app
                    nc.vector.tensor_add(oc[:, sl], OT[:], iac[:, sl])

                nc.sync.dma_start(outv[n], oc[:])

    nc.compile()
    return nc


def _get_nc():
    if "nc" not in _CACHE:
        _CACHE["nc"] = _build()
    return _CACHE["nc"]


def make_in_maps(i_app, hidden, Gm, bm, Gmax, Esyn):
    i_app = np.asarray(i_app, dtype=np.float32)
    hidden = np.asarray(hidden, dtype=np.float32)
    Gm_c = np.clip(np.asarray(Gm, np.float32), 0.01, 1.0)
    bm_c = np.clip(np.asarray(bm, np.float32), -1.0, 1.0)
    Gmax_c = np.clip(np.asarray(Gmax, np.float32), 0.0, 1.0)
    Esyn_c = np.clip(np.asarray(Esyn, np.float32), -3.0, 3.0)

    ge = np.sum(Gmax_c * Esyn_c, axis=0, dtype=np.float32)  # [N]

    params = {
        "negG": np.ascontiguousarray((-Gmax_c).astype(BF)),
        "identb": np.eye(N, dtype=BF),
        "ge": np.ascontiguousarray(ge.reshape(N, 1)),
        "omg": np.ascontiguousarray((1.0 - Gm_c).reshape(N, 1)),
        "bm": np.ascontiguousarray(bm_c.reshape(N, 1)),
    }
    in_maps = []
    for k in range(N_CORES):
        rows = slice(k * ROWS, (k + 1) * ROWS)
        in_maps.append(
            {
                "h": np.ascontiguousarray(hidden[rows]),
                "ia": np.ascontiguousarray(i_app[rows]),
                **params,
            }
        )
    return in_maps


def kernel(i_app, hidden, Gm, bm, Gmax, Esyn):
    nc = _get_nc()
    in_maps = make_in_maps(i_app, hidden, Gm, bm, Gmax, Esyn)
    res = run_bass_kernel_spmd(nc, in_maps, core_ids=list(range(N_CORES)))
    out = np.concatenate([res.results[k]["out"] for k in range(N_CORES)], axis=0)
    return (out, out)


# revision 10
# speedup vs baseline: 1.1052x; 1.1052x over previous
"""Trainium2 Bass kernel for nn_SNSCell (gnn_message_passing).

Math (per batch row b, feature j, n=128):
    Gm,bm,Gmax,Esyn are clipped; ge[j] = sum_i Gmax[i,j]*Esyn[i,j]
    P = h @ Gmax
    out[b,j] = (1-Gm[j])*h[b,j] + bm[j] + i_app[b,j]
               + clamp01(h[b,j]) * (ge[j] - P[b,j])

Strategy: data-parallel over batch across 8 cores (32768 rows each).
On-chip, work in the transposed domain (features on partitions) so
per-feature params are per-partition scalars and the matmul keeps a
fixed weight orientation:
  - h loaded with fp32->bf16 cast during DMA (SWDGE)
  - PE-transpose bf16 h blocks [128b,128i] -> hT [128i,128b]
  - PSUM Q = ge (K=2 hi/lo seed matmul) - Gmax^T-contraction with hT
  - elementwise on DVE (clamp/mul/fused scale-add), (1-Gm)/bm folded in
  - PE-transpose result back (bf16), add i_app in natural fp32, store.
"""

import numpy as np
import ml_dtypes
from contextlib import ExitStack

import concourse.bacc as bacc
import concourse.tile as tile
from concourse import mybir
from concourse.bass_utils import run_bass_kernel_spmd

B_FULL = 262144
N = 128
N_CORES = 8
ROWS = B_FULL // N_CORES          # 32768 rows per core
RPP = 16                          # rows packed per partition per DMA chunk
CHUNK_COLS = RPP * N              # 2048 cols per chunk tile
N_CHUNKS = ROWS // (128 * RPP)    # 16 chunks of [128, 2048] (1 MiB fp32)
SUPER = 512                       # cols per compute super-tile (= PSUM bank)
N_SUPER = CHUNK_COLS // SUPER     # 4 super-tiles per chunk

F32 = mybir.dt.float32
BF16 = mybir.dt.bfloat16
AOT = mybir.AluOpType
ACT_F = mybir.ActivationFunctionType
BF = ml_dtypes.bfloat16

_CACHE = {}


def _build():
    nc = bacc.Bacc("TRN2", debug=False)

    h = nc.dram_tensor("h", [ROWS, N], F32, kind="ExternalInput").ap()
    ia = nc.dram_tensor("ia", [ROWS, N], F32, kind="ExternalInput").ap()
    negG = nc.dram_tensor("negG", [N, N], BF16, kind="ExternalInput").ap()
    identb = nc.dram_tensor("identb", [N, N], BF16, kind="ExternalInput").ap()
    ge = nc.dram_tensor("ge", [N, 1], F32, kind="ExternalInput").ap()
    omg = nc.dram_tensor("omg", [N, 1], F32, kind="ExternalInput").ap()
    out = nc.dram_tensor("out", [ROWS, N], F32, kind="ExternalOutput").ap()

    hv = h.rearrange("(n p r) m -> n p (r m)", p=128, r=RPP)
    iav = ia.rearrange("(n p r) m -> n p (r m)", p=128, r=RPP)
    outv = out.rearrange("(n p r) m -> n p (r m)", p=128, r=RPP)

    with tile.TileContext(nc) as tc:
        with ExitStack() as ctx:
            const = ctx.enter_context(tc.tile_pool(name="const", bufs=1))
            io = ctx.enter_context(tc.tile_pool(name="io", bufs=2))
            mid = ctx.enter_context(tc.tile_pool(name="mid", bufs=4))
            ps = ctx.enter_context(tc.tile_pool(name="ps", bufs=3, space="PSUM"))
            psq = ctx.enter_context(tc.tile_pool(name="psq", bufs=2, space="PSUM"))

            negG_s = const.tile([N, N], BF16, tag="negG")
            ident_s = const.tile([N, N], BF16, tag="ident")
            ge_s = const.tile([N, 1], F32, tag="ge")
            omg_s = const.tile([N, 1], F32, tag="omg")
            nc.sync.dma_start(negG_s[:], negG[:])
            nc.sync.dma_start(ident_s[:], identb[:])
            nc.sync.dma_start(ge_s[:], ge[:])
            nc.sync.dma_start(omg_s[:], omg[:])

            for n in range(N_CHUNKS):
                hb = io.tile([128, CHUNK_COLS], BF16, tag="hb")
                iac = io.tile([128, CHUNK_COLS], BF16, tag="iac")
                oc = io.tile([128, CHUNK_COLS], F32, tag="oc")
                # cast fp32 -> bf16 during the load (SWDGE)
                nc.gpsimd.dma_start(hb[:], hv[n])
                nc.gpsimd.dma_start(iac[:], iav[n])

                for s in range(N_SUPER):
                    sl = slice(s * SUPER, (s + 1) * SUPER)

                    # hT blocks: [128 i, 512 b], bf16
                    T = ps.tile([128, SUPER], BF16, tag="T")
                    for r in range(4):
                        c0 = s * SUPER + r * 128
                        nc.tensor.transpose(
                            T[:, r * 128 : (r + 1) * 128],
                            hb[:, c0 : c0 + 128],
                            ident_s[:],
                        )
                    ht = mid.tile([128, SUPER], BF16, tag="ht")
                    nc.scalar.copy(ht[:], T[:])

                    # Q = -P^T
                    Q = psq.tile([128, SUPER], F32, tag="Q")
                    nc.tensor.matmul(Q[:], negG_s[:], ht[:], start=True, stop=True)

                    # t1 = ge - P^T   (ACT, PSUM src, per-partition bias)
                    t1 = mid.tile([128, SUPER], BF16, tag="t1")
                    nc.scalar.activation(
                        t1[:], Q[:], ACT_F.Identity, bias=ge_s[:], scale=1.0
                    )
                    # c = clamp01(hT)
                    cl = mid.tile([128, SUPER], BF16, tag="cl")
                    nc.vector.tensor_scalar(
                        cl[:], ht[:], 0.0, 1.0, AOT.max, AOT.min
                    )
                    # t = c * (ge - P^T)
                    t = mid.tile([128, SUPER], BF16, tag="t")
                    nc.vector.tensor_mul(t[:], cl[:], t1[:])
                    # u = (1-Gm)*hT + t   (bm folded into i_app host-side)
                    u = mid.tile([128, SUPER], BF16, tag="u")
                    nc.vector.scalar_tensor_tensor(
                        u[:], ht[:], omg_s[:], t[:], op0=AOT.mult, op1=AOT.add
                    )

                    # transpose back to natural layout
                    OT = ps.tile([128, SUPER], BF16, tag="OT")
                    for r in range(4):
                        rs = slice(r * 128, (r + 1) * 128)
                        nc.tensor.transpose(OT[:, rs], u[:, rs], ident_s[:])

                    # out = OT + i_app
                    nc.vector.tensor_add(oc[:, sl], OT[:], iac[:, sl])

                nc.sync.dma_start(outv[n], oc[:])

    nc.compile()
    return nc


def _get_nc():
    if "nc" not in _CACHE:
        _CACHE["nc"] = _build()
    return _CACHE["nc"]


def make_in_maps(i_app, hidden, Gm, bm, Gmax, Esyn):
    i_app = np.asarray(i_app, dtype=np.float32)
    hidden = np.asarray(hidden, dtype=np.float32)
    Gm_c = np.clip(np.asarray(Gm, np.float32), 0.01, 1.0)
    bm_c = np.clip(np.asarray(bm, np.float32), -1.0, 1.0)
    Gmax_c = np.clip(np.asarray(Gmax, np.float32), 0.0, 1.0)
    Esyn_c = np.clip(np.asarray(Esyn, np.float32), -3.0, 3.0)

    ge = np.sum(Gmax_c * Esyn_c, axis=0, dtype=np.float32)  # [N]

    params = {
        "negG": np.ascontiguousarray((-Gmax_c).astype(BF)),
        "identb": np.eye(N, dtype=BF),
        "ge": np.ascontiguousarray(ge.reshape(N, 1)),
        "omg": np.ascontiguousarray((1.0 - Gm_c).reshape(N, 1)),
    }
    ia_b = i_app + bm_c[None, :]  # fold bm into i_app
    in_maps = []
    for k in range(N_CORES):
        rows = slice(k * ROWS, (k + 1) * ROWS)
        in_maps.append(
            {
                "h": np.ascontiguousarray(hidden[rows]),
                "ia": np.ascontiguousarray(ia_b[rows]),
                **params,
            }
        )
    return in_maps


def kernel(i_app, hidden, Gm, bm, Gmax, Esyn):
    nc = _get_nc()
    in_maps = make_in_maps(i_app, hidden, Gm, bm, Gmax, Esyn)
    res = run_bass_kernel_spmd(nc, in_maps, core_ids=list(range(N_CORES)))
    out = np.concatenate([res.results[k]["out"] for k in range(N_CORES)], axis=0)
    return (out, out)
